# revision 23
# baseline (speedup 1.0000x reference)
"""2-layer GAT (100000 nodes, 32 neighbors) on 8 trn2 NeuronCores.

Strategy (SPMD, one Bass program for all 8 cores):
  - Nodes sharded 8 ways (12500/core). Small weights replicated, fused on
    host into one rhs per layer: [W | W@A2blk | W@A1blk] so one PE matmul
    per 128-node chunk emits h, s2=a2.h and s1=a1.h together.
  - fp32 node tables: layer-1 rows [h1(64)|s2(8)|s1(8)], layer-2 rows
    [h2(128)|s2(8)|s1(8)]. Each core builds only ITS 12500-row shard
    (98 chunk matmuls instead of 782 redundant ones); one AllGather per
    layer replicates the full table (collectives run on TOPSP+SDMA
    silicon, ~25us, and cost almost nothing).
  - Neighbor gathers: per-slot indirect DMAs (32 per 128-dst tile; the
    dst's own s1 is stashed in SBUF during the table builds, so no slot-0
    gather). The HW honors exactly one dynamic offset per partition per
    SWDGE instruction (~1.2us each, Pool-engine bound) -- this is the
    kernel's wall.
  - The exp(logit) tensor is broadcast-expanded over the head width on
    the ACT engine (frees DVE); the 32-neighbor reduction is a pairwise
    add tree; softmax normalization is fused scalar work. All hidden
    under the gather wall.
  - Layer-2 table rows are produced inline in the layer-1 loop
    (x2 -> PE transpose -> matmul rhs2) so no separate full-table pass.
Output: per-core [12500,16] fp32 shard, concatenated on the host.
"""
import sys

if '/opt/trn_rl_repo' not in sys.path:
    sys.path.insert(0, '/opt/trn_rl_repo')

import numpy as np
import concourse.bass as bass
import concourse.bacc as bacc
import concourse.mybir as mybir
from concourse.tile import TileContext
from concourse.masks import make_identity

import jax
from jax.sharding import Mesh, PartitionSpec
from jax.experimental.shard_map import shard_map
from concourse.bass2jax import (_bass_exec_p, install_neuronx_cc_hook,
                                partition_id_tensor)

FP = mybir.dt.float32
F16 = mybir.dt.float16
I32 = mybir.dt.int32
AF = mybir.ActivationFunctionType
OP = mybir.AluOpType
AX = mybir.AxisListType

N_NODES = 100000
N_CORES = 8
D_NBR = 32
K1, F1 = 8, 8
K2, F2 = 8, 16
NEG_SLOPE = 0.01
TB = 1                                   # destination tiles per gather batch

H1, H2 = K1 * F1, K2 * F2                # 64, 128
R1 = H1 + 8 + 8                          # 80  : [h1 | s2 | s1]
R2 = H2 + 8 + 8                          # 144 : [h2 | s2 | s1]
NI = D_NBR                               # neighbor slots only; dst s1 is stashed in SBUF


def _emit_gat(nc, xTs, rhs1, rhs2, nbrp, out, N, n_cores):
    """Emit the whole per-core program. All APs are DRAM tensors."""
    S = N // n_cores
    n_tiles = (S + 127) // 128
    n_iters = (n_tiles + TB - 1) // TB
    D = D_NBR

    t1_bounce = nc.dram_tensor("t1_bounce", [S, R1], FP).ap()
    table1 = nc.dram_tensor("table1", [N, R1], FP, addr_space="Shared").ap()
    t2_bounce = nc.dram_tensor("t2_bounce", [S, R2], FP).ap()
    table2 = nc.dram_tensor("table2", [N, R2], FP, addr_space="Shared").ap()

    with TileContext(nc) as tc:
        with tc.tile_pool(name="const", bufs=1) as cp, \
             tc.tile_pool(name="psum", bufs=2, space="PSUM") as pp:

            rt1 = cp.tile([128, R1], FP)
            nc.sync.dma_start(out=rt1[:], in_=rhs1[:, :])
            rt2 = cp.tile([H1, R2], FP)
            nc.sync.dma_start(out=rt2[:], in_=rhs2[:, :])
            ident = cp.tile([128, 128], FP)
            make_identity(nc, ident[:])
            idx = cp.tile([128, n_iters * TB * NI], I32)
            nc.sync.dma_start(out=idx[:], in_=nbrp[:, :])
            s1own1 = cp.tile([128, n_tiles * K1], FP)
            s1own2 = cp.tile([128, n_tiles * K2], FP)

            # ---- T1: this core's shard of the layer-1 table ----
            with tc.tile_pool(name="t1", bufs=2) as tp, \
                 tc.tile_pool(name="xts", bufs=1) as xp:
                xt = xp.tile([128, n_tiles * 128], FP)
                nc.sync.dma_start(out=xt[:], in_=xTs[:, :])
                for c in range(n_tiles):
                    g0 = c * 128
                    M = min(128, S - g0)
                    ps = pp.tile([128, R1], FP, name=f"t1p{c}", tag="t1p",
                                 space="PSUM")
                    nc.tensor.matmul(out=ps[:], lhsT=xt[:, g0:g0 + 128],
                                     rhs=rt1[:], start=True, stop=True)
                    row = tp.tile([128, R1], FP, name=f"t1r{c}", tag="t1r")
                    nc.vector.tensor_copy(out=row[:], in_=ps[:])
                    nc.vector.tensor_copy(
                        out=s1own1[:, c * K1:(c + 1) * K1],
                        in_=ps[:, H1 + 8:H1 + 16])
                    nc.sync.dma_start(out=t1_bounce[g0:g0 + M, :],
                                      in_=row[:M, :])

            nc.gpsimd.collective_compute(
                "AllGather", OP.bypass,
                replica_groups=[list(range(n_cores))],
                ins=[t1_bounce.opt()], outs=[table1.opt()])

            # ---- A1 + fused layer-2 table build ----
            with tc.tile_pool(name="a1", bufs=4) as ap_:
                for i in range(n_iters):
                    t0 = i * TB
                    hg = ap_.tile([128, TB * NI * R1], FP, name=f"g{i}",
                                  tag="hg")
                    hgv = hg[:].rearrange("p (t n r) -> p t n r", n=NI, r=R1)
                    for jj in range(TB * NI):
                        nc.gpsimd.indirect_dma_start(
                            out=hgv[:, jj // NI, jj % NI, :],
                            out_offset=None, in_=table1[:],
                            in_offset=bass.IndirectOffsetOnAxis(
                                ap=idx[:, t0 * NI + jj:t0 * NI + jj + 1],
                                axis=0))
                    # e[p,t,d,k] = s1(dst) + s2(nbr); leaky_relu in place
                    e = ap_.tile([128, TB * D * K1], FP, name=f"e{i}",
                                 tag="e")
                    ev = e[:].rearrange("p (t d k) -> p t d k", d=D, k=K1)
                    nc.vector.tensor_tensor(
                        out=ev, in0=hgv[:, :, :, H1:H1 + 8],
                        in1=s1own1[:, t0 * K1:(t0 + TB) * K1]
                            .rearrange("p (t k) -> p t k", k=K1)
                            .unsqueeze(2).to_broadcast([128, TB, D, K1]),
                        op=OP.add)
                    el = ap_.tile([128, TB * D * K1], FP, name=f"l{i}",
                                  tag="el")
                    nc.vector.scalar_tensor_tensor(
                        out=el[:], in0=e[:], scalar=NEG_SLOPE, in1=e[:],
                        op0=OP.mult, op1=OP.max)
                    # u (for z) and broadcast-expanded exp(e) (for the sum)
                    us = ap_.tile([128, TB * D * K1], FP, name=f"u{i}",
                                  tag="us")
                    nc.scalar.activation(out=us[:], in_=el[:], func=AF.Exp)
                    uf = ap_.tile([128, TB * D * H1], FP, name=f"f{i}",
                                  tag="uf")
                    nc.scalar.activation(
                        out=uf[:].rearrange("p (td k f) -> p td k f",
                                            k=K1, f=F1),
                        in_=el[:].rearrange("p (td k) -> p td k", k=K1)
                            .unsqueeze(3).to_broadcast([128, TB * D, K1, F1]),
                        func=AF.Exp)
                    z = ap_.tile([128, TB * K1], FP, name=f"z{i}", tag="z")
                    nc.vector.tensor_reduce(
                        out=z[:].rearrange("p (t k) -> p t k", k=K1),
                        in_=us[:].rearrange("p (t d k) -> p t d k", d=D, k=K1)
                            .transpose([0, 1, 3, 2]),
                        axis=AX.X, op=OP.add)
                    rz = ap_.tile([128, TB * K1], FP, name=f"r{i}", tag="rz")
                    nc.vector.reciprocal(out=rz[:], in_=z[:])
                    # weighted sum over neighbors: mult at 2x + add tree
                    tmp = ap_.tile([128, TB * D * H1], FP, name=f"m{i}",
                                   tag="tmp")
                    tv = tmp[:].rearrange("p (t d h) -> p t d h", d=D, h=H1)
                    fv = uf[:].rearrange("p (t d h) -> p t d h", d=D, h=H1)
                    nc.vector.tensor_tensor(
                        out=tv, in0=hgv[:, :, :, 0:H1], in1=fv, op=OP.mult)
                    cur, oth = tv, fv
                    w = D // 2
                    while w >= 1:
                        nc.vector.tensor_tensor(
                            out=oth[:, :, 0:w, :], in0=cur[:, :, 0:w, :],
                            in1=cur[:, :, w:2 * w, :], op=OP.add)
                        cur, oth = oth, cur
                        w //= 2
                    s = cur[:, :, 0:1, :].rearrange(
                        "p t u (k f) -> p (t u) k f", f=F1)
                    o = ap_.tile([128, TB * H1], FP, name=f"o{i}", tag="o")
                    nc.vector.tensor_tensor(
                        out=o[:].rearrange("p (t k f) -> p t k f", k=K1,
                                           f=F1),
                        in0=s,
                        in1=rz[:].rearrange("p (t k) -> p t k", k=K1)
                            .unsqueeze(3).to_broadcast([128, TB, K1, F1]),
                        op=OP.mult)
                    # elu(o) = max(o, exp(min(o,0)) - 1)
                    mn = ap_.tile([128, TB * H1], FP, name=f"n{i}", tag="mn")
                    nc.vector.tensor_scalar_min(out=mn[:], in0=o[:],
                                                scalar1=0.0)
                    nc.scalar.activation(out=mn[:], in_=mn[:], func=AF.Exp)
                    x2 = ap_.tile([128, TB * H1], FP, name=f"x{i}", tag="x2")
                    nc.vector.scalar_tensor_tensor(
                        out=x2[:], in0=mn[:], scalar=-1.0, in1=o[:],
                        op0=OP.add, op1=OP.max)
                    # layer-2 table rows for these tiles
                    row2 = ap_.tile([128, TB * R2], FP, name=f"w{i}",
                                    tag="row2")
                    r2v = row2[:].rearrange("p (t r) -> p t r", r=R2)
                    for tt in range(TB):
                        if t0 + tt >= n_tiles:
                            break
                        psT = pp.tile([H1, 128], FP, name=f"pt{i}_{tt}",
                                      tag="psT", space="PSUM")
                        nc.tensor.transpose(
                            out=psT[:],
                            in_=x2[:].rearrange("p (t h) -> p t h", h=H1)
                            [:, tt, :],
                            identity=ident[:])
                        x2T = ap_.tile([H1, 128], FP, name=f"y{i}_{tt}",
                                       tag="x2T")
                        nc.vector.tensor_copy(out=x2T[:], in_=psT[:])
                        ps2 = pp.tile([128, R2], FP, name=f"p2{i}_{tt}",
                                      tag="ps2", space="PSUM")
                        nc.tensor.matmul(out=ps2[:], lhsT=x2T[:], rhs=rt2[:],
                                         start=True, stop=True)
                        nc.vector.tensor_copy(out=r2v[:, tt, :], in_=ps2[:])
                        nc.vector.tensor_copy(
                            out=s1own2[:, (t0 + tt) * K2:(t0 + tt + 1) * K2],
                            in_=ps2[:, H2 + 8:H2 + 16])
                        g0 = (t0 + tt) * 128
                        M = min(128, S - g0)
                        nc.sync.dma_start(out=t2_bounce[g0:g0 + M, :],
                                          in_=r2v[:M, tt, :])

            nc.gpsimd.collective_compute(
                "AllGather", OP.bypass,
                replica_groups=[list(range(n_cores))],
                ins=[t2_bounce.opt()], outs=[table2.opt()])

            # ---- A2: layer-2 attention + head mean + softmax ----
            with tc.tile_pool(name="a2", bufs=3) as bp:
                for i in range(n_iters):
                    t0 = i * TB
                    hg = bp.tile([128, TB * NI * R2], FP, name=f"G{i}",
                                 tag="hg2")
                    hgv = hg[:].rearrange("p (t n r) -> p t n r", n=NI, r=R2)
                    for jj in range(TB * NI):
                        nc.gpsimd.indirect_dma_start(
                            out=hgv[:, jj // NI, jj % NI, :],
                            out_offset=None, in_=table2[:],
                            in_offset=bass.IndirectOffsetOnAxis(
                                ap=idx[:, t0 * NI + jj:t0 * NI + jj + 1],
                                axis=0))
                    e = bp.tile([128, TB * D * K2], FP, name=f"E{i}",
                                tag="e2")
                    ev = e[:].rearrange("p (t d k) -> p t d k", d=D, k=K2)
                    nc.vector.tensor_tensor(
                        out=ev, in0=hgv[:, :, :, H2:H2 + 8],
                        in1=s1own2[:, t0 * K2:(t0 + TB) * K2]
                            .rearrange("p (t k) -> p t k", k=K2)
                            .unsqueeze(2).to_broadcast([128, TB, D, K2]),
                        op=OP.add)
                    el = bp.tile([128, TB * D * K2], FP, name=f"L{i}",
                                 tag="el2")
                    nc.vector.scalar_tensor_tensor(
                        out=el[:], in0=e[:], scalar=NEG_SLOPE, in1=e[:],
                        op0=OP.mult, op1=OP.max)
                    us = bp.tile([128, TB * D * K2], FP, name=f"U{i}",
                                 tag="us2")
                    nc.scalar.activation(out=us[:], in_=el[:], func=AF.Exp)
                    uf = bp.tile([128, TB * D * H2], FP, name=f"F{i}",
                                 tag="uf2")
                    nc.scalar.activation(
                        out=uf[:].rearrange("p (td k f) -> p td k f",
                                            k=K2, f=F2),
                        in_=el[:].rearrange("p (td k) -> p td k", k=K2)
                            .unsqueeze(3).to_broadcast([128, TB * D, K2, F2]),
                        func=AF.Exp)
                    z = bp.tile([128, TB * K2], FP, name=f"Z{i}", tag="z2")
                    nc.vector.tensor_reduce(
                        out=z[:].rearrange("p (t k) -> p t k", k=K2),
                        in_=us[:].rearrange("p (t d k) -> p t d k", d=D,
                                            k=K2).transpose([0, 1, 3, 2]),
                        axis=AX.X, op=OP.add)
                    rz = bp.tile([128, TB * K2], FP, name=f"R{i}", tag="rz2")
                    nc.vector.reciprocal(out=rz[:], in_=z[:])
                    tmp = bp.tile([128, TB * D * H2], FP, name=f"M{i}",
                                  tag="tmp2")
                    tv = tmp[:].rearrange("p (t d h) -> p t d h", d=D, h=H2)
                    fv = uf[:].rearrange("p (t d h) -> p t d h", d=D, h=H2)
                    nc.vector.tensor_tensor(
                        out=tv, in0=hgv[:, :, :, 0:H2], in1=fv, op=OP.mult)
                    cur, oth = tv, fv
                    w = D // 2
                    while w >= 1:
                        nc.vector.tensor_tensor(
                            out=oth[:, :, 0:w, :], in0=cur[:, :, 0:w, :],
                            in1=cur[:, :, w:2 * w, :], op=OP.add)
                        cur, oth = oth, cur
                        w //= 2
                    s = cur[:, :, 0:1, :].rearrange(
                        "p t u (k f) -> p (t u) k f", f=F2)
                    o = bp.tile([128, TB * H2], FP, name=f"O{i}", tag="o2")
                    ov = o[:].rearrange("p (t k f) -> p t k f", k=K2, f=F2)
                    nc.vector.tensor_tensor(
                        out=ov, in0=s,
                        in1=rz[:].rearrange("p (t k) -> p t k", k=K2)
                            .unsqueeze(3).to_broadcast([128, TB, K2, F2]),
                        op=OP.mult)
                    # mean over heads (scaled inside the exp) + softmax
                    mo = bp.tile([128, TB * F2], FP, name=f"Q{i}", tag="mo")
                    nc.vector.tensor_reduce(
                        out=mo[:].rearrange("p (t f) -> p t f", f=F2),
                        in_=o[:].rearrange("p (t k f) -> p t k f", k=K2,
                                           f=F2).transpose([0, 1, 3, 2]),
                        axis=AX.X, op=OP.add)
                    u3 = bp.tile([128, TB * F2], FP, name=f"V{i}", tag="u3")
                    z3 = bp.tile([128, TB], FP, name=f"W{i}", tag="z3")
                    u3v = u3[:].rearrange("p (t f) -> p t f", f=F2)
                    mov = mo[:].rearrange("p (t f) -> p t f", f=F2)
                    for tt in range(TB):
                        nc.scalar.activation(
                            out=u3v[:, tt, :], in_=mov[:, tt, :], func=AF.Exp,
                            scale=1.0 / K2, accum_out=z3[:, tt:tt + 1])
                    rz3 = bp.tile([128, TB], FP, name=f"X{i}", tag="rz3")
                    nc.vector.reciprocal(out=rz3[:], in_=z3[:])
                    ot = bp.tile([128, TB * F2], FP, name=f"Y{i}", tag="ot")
                    otv = ot[:].rearrange("p (t f) -> p t f", f=F2)
                    nc.vector.tensor_tensor(
                        out=otv, in0=u3v,
                        in1=rz3[:].unsqueeze(2).to_broadcast([128, TB, F2]),
                        op=OP.mult)
                    for tt in range(TB):
                        if t0 + tt >= n_tiles:
                            break
                        g0 = (t0 + tt) * 128
                        M = min(128, S - g0)
                        nc.sync.dma_start(out=out[g0:g0 + M, :],
                                          in_=otv[:M, tt, :])
    return nc


def _build_gat(N=N_NODES, n_cores=N_CORES):
    S = N // n_cores
    n_tiles = (S + 127) // 128
    n_iters = (n_tiles + TB - 1) // TB

    nc = bacc.Bacc("TRN2", target_bir_lowering=False, debug=False,
                   num_devices=n_cores)
    xTs = nc.dram_tensor("xTs", [128, n_tiles * 128], FP,
                         kind="ExternalInput").ap()
    rhs1 = nc.dram_tensor("rhs1", [128, R1], FP, kind="ExternalInput").ap()
    rhs2 = nc.dram_tensor("rhs2", [H1, R2], FP, kind="ExternalInput").ap()
    nbrp = nc.dram_tensor("nbrp", [128, n_iters * TB * NI], I32,
                          kind="ExternalInput").ap()
    out = nc.dram_tensor("out", [S, F2], FP, kind="ExternalOutput").ap()
    _emit_gat(nc, xTs, rhs1, rhs2, nbrp, out, N, n_cores)
    nc.finalize()
    return nc


class _SpmdRunner:
    """jit-once SPMD executor over the 8 axon NeuronCores."""

    def __init__(self, nc, n_cores):
        install_neuronx_cc_hook()
        self.nc, self.n_cores = nc, n_cores
        partition_name = (nc.partition_id_tensor.name
                          if nc.partition_id_tensor else None)
        in_names, out_names, out_avals, zero_outs = [], [], [], []
        for alloc in nc.m.functions[0].allocations:
            if not isinstance(alloc, mybir.MemoryLocationSet):
                continue
            name = alloc.memorylocations[0].name
            if alloc.kind == "ExternalInput":
                if name != partition_name:
                    in_names.append(name)
            elif alloc.kind == "ExternalOutput":
                out_names.append(name)
                shape = tuple(alloc.tensor_shape)
                dtype = mybir.dt.np(alloc.dtype)
                out_avals.append(jax.core.ShapedArray(shape, dtype))
                zero_outs.append(np.zeros(shape, dtype))
        self.in_names, self.out_names = in_names, out_names
        self.out_avals, self.zero_outs = out_avals, zero_outs
        all_in_names = in_names + out_names
        if partition_name is not None:
            all_in_names.append(partition_name)

        def _body(*args):
            operands = list(args)
            if partition_name is not None:
                operands.append(partition_id_tensor())
            return tuple(_bass_exec_p.bind(
                *operands, out_avals=tuple(out_avals),
                in_names=tuple(all_in_names), out_names=tuple(out_names),
                lowering_input_output_aliases=(),
                sim_require_finite=True, sim_require_nnan=True, nc=nc))

        devices = jax.devices()[:n_cores]
        self.mesh = Mesh(np.asarray(devices), ("core",))
        n_params, n_outs = len(in_names), len(out_avals)
        in_specs = (PartitionSpec("core"),) * (n_params + n_outs)
        out_specs = (PartitionSpec("core"),) * n_outs
        self.fn = jax.jit(
            shard_map(_body, mesh=self.mesh, in_specs=in_specs,
                      out_specs=out_specs, check_rep=False),
            keep_unused=True)
        self.sharding = jax.sharding.NamedSharding(self.mesh,
                                                   PartitionSpec("core"))

    def run(self, in_maps):
        per_core = [[np.asarray(m[n]) for n in self.in_names] for m in in_maps]
        concat = [np.concatenate([per_core[c][i] for c in range(self.n_cores)],
                                 axis=0) for i in range(len(self.in_names))]
        zeros = [np.zeros((self.n_cores * z.shape[0], *z.shape[1:]), z.dtype)
                 for z in self.zero_outs]
        dev = [jax.device_put(a, self.sharding) for a in concat + zeros]
        outs = self.fn(*dev)
        jax.block_until_ready(outs)
        res = []
        for c in range(self.n_cores):
            res.append({name: np.asarray(outs[i]).reshape(
                self.n_cores, *self.out_avals[i].shape)[c]
                for i, name in enumerate(self.out_names)})
        return res


def _blk(a, k, f):
    A = np.zeros((k * f, k), np.float32)
    for kk in range(k):
        A[kk * f:(kk + 1) * f, kk] = a[kk]
    return A


def _host_prep(node_features, neighbors, W1, a1_1, a2_1, W2, a1_2, a2_2):
    N = node_features.shape[0]
    S = N // N_CORES
    n_tiles = (S + 127) // 128
    n_iters = (n_tiles + TB - 1) // TB

    # column order [W | W@A2blk | W@A1blk] -> table rows [h | s2 | s1]
    rhs1 = np.concatenate(
        [W1, W1 @ _blk(a2_1, K1, F1), W1 @ _blk(a1_1, K1, F1)],
        axis=1).astype(np.float32)
    rhs2 = np.concatenate(
        [W2, W2 @ _blk(a2_2, K2, F2), W2 @ _blk(a1_2, K2, F2)],
        axis=1).astype(np.float32)
    xT = np.ascontiguousarray(node_features.T).astype(np.float32)

    in_maps = []
    for r in range(N_CORES):
        xTs = np.zeros((128, n_tiles * 128), np.float32)
        xTs[:, :S] = xT[:, r * S:(r + 1) * S]
        nb = neighbors[r * S:(r + 1) * S].astype(np.int32)  # [S, 32]
        nbp = np.zeros((n_iters * TB * 128, NI), np.int32)
        nbp[:S] = nb
        # [tile, p, j] -> [p, tile*j] so tile t occupies idx cols t*NI..
        nbrp = np.ascontiguousarray(
            nbp.reshape(n_iters * TB, 128, NI).transpose(1, 0, 2)
            .reshape(128, -1))
        in_maps.append({'xTs': xTs, 'rhs1': rhs1, 'rhs2': rhs2,
                        'nbrp': nbrp})
    return in_maps


_RUNNER = None


def _get_runner():
    global _RUNNER
    if _RUNNER is None:
        nc = _build_gat()
        _RUNNER = _SpmdRunner(nc, N_CORES)
    return _RUNNER


def kernel(node_features, neighbors, W1, a1_1, a2_1, W2, a1_2, a2_2):
    node_features = np.asarray(node_features, dtype=np.float32)
    neighbors = np.asarray(neighbors)
    runner = _get_runner()
    in_maps = _host_prep(node_features, neighbors,
                         np.asarray(W1, np.float32),
                         np.asarray(a1_1, np.float32),
                         np.asarray(a2_1, np.float32),
                         np.asarray(W2, np.float32),
                         np.asarray(a1_2, np.float32),
                         np.asarray(a2_2, np.float32))
    res = runner.run(in_maps)
    S = node_features.shape[0] // N_CORES
    return np.concatenate([res[c]['out'][:S] for c in range(N_CORES)], axis=0)
